# revision 4
# baseline (speedup 1.0000x reference)
"""Trainium2 Bass kernel for nn_BinaryEncoding (per-position top-16 mask
along the 256-filter dim of [32, 256, 56, 56] activations).

Algorithm (exact threshold selection, DVE-light):
  per position row (256 channel values in the free dim):
    max8 -> top-8 values m1 (descending), t8 = m1[7];
    v = (x < t8) * x          (fused scalar_tensor_tensor: zeroes the top-8;
                               the zeros rank below x_(16) since x_(16) > 0
                               for 256 N(0,1) samples w.p. ~1)
    max8(v) -> m2, t16 = m2[7] = 16th largest of x (exact f32 value);
    mask = (x >= t16)          (fused tensor_scalar is_ge -> {0,1}, bf16 out)

  DVE work/block: 2x max8 + 2x fused tensor-scalar (the latter run in the
  2x_2p all-SBUF fast mode) ~= 1.05us vs 3.1us for the old
  max8/match_replace x2 pipeline. The per-block DVE ops are emitted with a
  4-deep cross-block stagger so no DVE op consumes a result produced by the
  immediately preceding DVE op (result-forwarding stall avoidance).

Layout: HBM in is channel-major [img, 256, 3136] f32; tiles are DMA'd as
  [128 ch, pos] (contiguous), transposed on TensorE via identity matmul into
  PSUM [128 pos, 256 ch], copied to SBUF by ScalarE. The mask is stored
  pos-major [pos, 256] bf16 (no output transposes; one batched DMA per
  chunk); the host casts to f32 and transposes back to channel-major.

Sharding: pure data parallel, 4 images per core across 8 cores.
"""

import numpy as np

import concourse.bacc as bacc
import concourse.bass as bass
import concourse.mybir as mybir
from concourse import tile
from concourse.bass_utils import run_bass_kernel_spmd
from concourse.masks import make_identity

P = 128
C = 256                      # filter dim
N_CORES = 8


def _segments(s, e, hw):
    """Split flat-position range [s, e) into per-image contiguous pieces.

    Returns [(img, h0, h1, off)] with off the offset inside the chunk."""
    res = []
    off = 0
    while s < e:
        img = s // hw
        h0 = s - img * hw
        h1 = min(e - img * hw, hw)
        res.append((img, h0, h1, off))
        off += h1 - h0
        s = img * hw + h1
    return res


def _strip_self_waits(nc, engines=("DVE",)):
    """Remove semaphore waits where an instruction waits on its OWN
    engine's semaphore. Engines execute their stream in order, so a wait
    on a value that only earlier same-engine instructions increment is
    always satisfied; it just adds a sem round-trip to every dispatch."""
    n = 0
    for blk in nc.m.functions[0].blocks:
        for inst in blk.instructions:
            eng = str(getattr(inst, "engine", ""))
            si = getattr(inst, "sync_info", None)
            if si is None or not si.on_wait:
                continue
            eng_name = eng.split(".")[-1]
            if eng_name not in engines:
                continue
            keep = [w for w in si.on_wait
                    if not (w.ant_name or "").startswith(eng_name + "_")]
            if len(keep) != len(si.on_wait):
                n += len(si.on_wait) - len(keep)
                si.on_wait = keep
    return n


def _plan(nblk, chunk_blocks, taper):
    if taper and nblk >= 24:
        # small first/last chunks shrink the DMA ramp at kernel start/end
        plan = [2, 4]
        while sum(plan) + chunk_blocks <= nblk - 6:
            plan.append(chunk_blocks)
        rem = nblk - sum(plan)
        if rem > 4:
            plan.extend([rem - 2, 2])
        elif rem > 0:
            plan.append(rem)
    else:
        assert nblk % chunk_blocks == 0
        plan = [chunk_blocks] * (nblk // chunk_blocks)
    assert sum(plan) == nblk
    return plan


def build_nc(n_img=4, hw=3136, chunk_blocks=14, in_bufs=3, out_bufs=3,
             x_bufs=7, v_bufs=4, m_bufs=6, ps_bufs=4, taper=True,
             out_dt="bf16", s2_gp_mod=0, s4_gp_mod=0,
             strip_self_waits=()):
    tot = n_img * hw
    assert tot % P == 0
    nblk = tot // P
    plan = _plan(nblk, chunk_blocks, taper)
    f32 = mybir.dt.float32
    odt = {"bf16": mybir.dt.bfloat16, "fp8": mybir.dt.float8e4,
           "f32": f32}[out_dt]

    nc = bacc.Bacc("TRN2", target_bir_lowering=False, debug=False,
                   num_devices=N_CORES)
    x = nc.declare_dram_parameter("x", [n_img, C, hw], f32, isOutput=False)
    y = nc.declare_dram_parameter("y", [nblk, P, C], odt, isOutput=True)

    # chunk id / index-in-chunk / chunk start block for each global block
    chunk_of, j_of, c0_of = [], [], []
    b0 = 0
    for ci, cb in enumerate(plan):
        for j in range(cb):
            chunk_of.append(ci)
            j_of.append(j)
            c0_of.append(b0)
        b0 += cb

    with tile.TileContext(nc) as tc:
        with (
            tc.tile_pool(name="const", bufs=1) as const_pool,
            tc.tile_pool(name="inp", bufs=in_bufs) as in_pool,
            tc.tile_pool(name="outp", bufs=out_bufs) as out_pool,
            tc.tile_pool(name="xx", bufs=x_bufs) as x_pool,
            tc.tile_pool(name="vv", bufs=v_bufs) as v_pool,
            tc.tile_pool(name="m8", bufs=2 * m_bufs) as m_pool,
            tc.tile_pool(name="psin", bufs=ps_bufs, space="PSUM") as ps_pool,
        ):
            ident = const_pool.tile([P, P], f32)
            make_identity(nc, ident)

            ctx = {}        # per-inflight-block state, keyed by global blk
            chunk_tiles = {}

            def s0(g):
                ci = chunk_of[g]
                if j_of[g] == 0:
                    cb = plan[ci]
                    s = c0_of[g] * P
                    in_lo = in_pool.tile([P, cb * P], f32, tag="in_lo")
                    in_hi = in_pool.tile([P, cb * P], f32, tag="in_hi")
                    for (img, h0, h1, off) in _segments(s, s + cb * P, hw):
                        n = h1 - h0
                        nc.sync.dma_start(out=in_lo[:, off:off + n],
                                          in_=x[img, 0:P, h0:h1])
                        nc.sync.dma_start(out=in_hi[:, off:off + n],
                                          in_=x[img, P:C, h0:h1])
                    outc = out_pool.tile([P, cb, C], odt, tag="outc")
                    chunk_tiles[ci] = (in_lo, in_hi, outc)
                in_lo, in_hi, outc = chunk_tiles[ci]
                j = j_of[g]
                sl = slice(j * P, (j + 1) * P)
                ps = ps_pool.tile([P, C], f32, tag="ps", name="ps")
                nc.tensor.transpose(ps[:, 0:P], in_lo[:, sl], ident)
                nc.tensor.transpose(ps[:, P:C], in_hi[:, sl], ident)
                xsb = x_pool.tile([P, C], f32, tag="x", name="x")
                nc.scalar.activation(xsb, ps,
                                     mybir.ActivationFunctionType.Copy)
                ctx[g] = {"x": xsb}

            def s1(g):
                b = ctx[g]
                b["m1"] = m_pool.tile([P, 8], f32, tag="m1", name="m1")
                nc.vector.max(out=b["m1"], in_=b["x"])

            def s2(g):
                b = ctx[g]
                b["v"] = v_pool.tile([P, C], f32, tag="v", name="v")
                eng = (nc.gpsimd if s2_gp_mod and g % s2_gp_mod == 0
                       else nc.vector)
                eng.scalar_tensor_tensor(
                    out=b["v"], in0=b["x"], scalar=b["m1"][:, 7:8],
                    in1=b["x"], op0=mybir.AluOpType.is_lt,
                    op1=mybir.AluOpType.mult)

            def s3(g):
                b = ctx[g]
                b["m2"] = m_pool.tile([P, 8], f32, tag="m2", name="m2")
                nc.vector.max(out=b["m2"], in_=b["v"])

            def s4(g):
                b = ctx[g]
                ci = chunk_of[g]
                outc = chunk_tiles[ci][2]
                eng = (nc.gpsimd if s4_gp_mod and g % s4_gp_mod == 0
                       else nc.vector)
                eng.tensor_scalar(out=outc[:, j_of[g], :], in0=b["x"],
                                  scalar1=b["m2"][:, 7:8], scalar2=None,
                                  op0=mybir.AluOpType.is_ge)
                if j_of[g] == plan[ci] - 1:
                    cb = plan[ci]
                    b0 = c0_of[g]
                    nc.sync.dma_start(
                        out=y[b0:b0 + cb].rearrange("b p c -> p b c"),
                        in_=outc[:, 0:cb, :])
                del ctx[g]

            stages = [s0, s1, s2, s3, s4]
            for step in range(nblk + 4):
                for d, st in enumerate(stages):
                    g = step - d
                    if 0 <= g < nblk:
                        st(g)
    nc.compile()
    if strip_self_waits:
        _strip_self_waits(nc, tuple(strip_self_waits))
    return nc


def _install_neff_cache():
    """Cache compiled NEFFs by BIR hash under /tmp so repeat runs skip
    the multi-minute neuronxcc compile."""
    import hashlib
    import os
    import shutil
    import concourse.bass2jax as b2j
    if getattr(b2j, "_topk_neff_cache_installed", False):
        return
    cache_dir = "/tmp/neff_cache"
    try:
        os.makedirs(cache_dir, exist_ok=True)
    except OSError:
        return
    orig_compile = b2j.compile_bir_kernel

    def cached_compile(ant_bir_str, compile_dir_path, neff_name):
        key = hashlib.sha256(ant_bir_str).hexdigest()[:32]
        cpath = os.path.join(cache_dir, key + ".neff")
        if os.path.exists(cpath):
            dst = os.path.join(compile_dir_path, neff_name)
            shutil.copy(cpath, dst)
            return dst
        out = orig_compile(ant_bir_str, compile_dir_path, neff_name=neff_name)
        try:
            shutil.copy(out, cpath)
        except OSError:
            pass
        return out

    b2j.compile_bir_kernel = cached_compile
    b2j._topk_neff_cache_installed = True


_install_neff_cache()

_NC_CACHE = {}


def _get_nc(n_img, hw, chunk_blocks, **kw):
    key = (n_img, hw, chunk_blocks, tuple(sorted(kw.items())))
    if key not in _NC_CACHE:
        _NC_CACHE[key] = build_nc(n_img, hw, chunk_blocks, **kw)
    return _NC_CACHE[key]


KERNEL_KW = dict(strip_self_waits=("DVE",))


def make_in_maps(x, n_img, kw=KERNEL_KW):
    return [{"x": np.ascontiguousarray(x[i * n_img:(i + 1) * n_img])}
            for i in range(N_CORES)]


def kernel(activations: np.ndarray) -> np.ndarray:
    B, Cin, H, W = activations.shape
    assert (B, Cin, H, W) == (32, 256, 56, 56)
    hw = H * W
    n_img = B // N_CORES
    x = np.ascontiguousarray(activations, dtype=np.float32).reshape(B, Cin, hw)
    nc = _get_nc(n_img, hw, 14, **KERNEL_KW)
    in_maps = make_in_maps(x, n_img)
    res = run_bass_kernel_spmd(nc, in_maps, list(range(N_CORES)))
    parts = []
    for i in range(N_CORES):
        yc = np.asarray(res.results[i]["y"])         # [nblk, 128, C] bf16
        yc = yc.reshape(n_img, hw, Cin).astype(np.float32)
        parts.append(yc.transpose(0, 2, 1))          # -> [n_img, C, hw]
    y = np.concatenate(parts, axis=0)
    return np.ascontiguousarray(y.reshape(B, Cin, H, W))


# revision 9
# speedup vs baseline: 1.2034x; 1.2034x over previous
"""Trainium2 Bass kernel for nn_BinaryEncoding (per-position top-16 mask
along the 256-filter dim of [32, 256, 56, 56] activations).

Algorithm (exact threshold selection, DVE-light):
  per position row (256 channel values in the free dim):
    max8 -> top-8 values m1 (descending), t8 = m1[7];
    v = (x < t8) * x          (fused scalar_tensor_tensor: zeroes the top-8;
                               the zeros rank below x_(16) since x_(16) > 0
                               for 256 N(0,1) samples w.p. ~1)
    max8(v) -> m2, t16 = m2[7] = 16th largest of x (exact f32 value);
    mask = (x >= t16)          (fused tensor_scalar is_ge -> {0,1}, bf16 out)

  DVE work/block: 2x max8 + 2x fused tensor-scalar (the latter run in the
  2x_2p all-SBUF fast mode) ~= 1.05us vs 3.1us for the old
  max8/match_replace x2 pipeline. The per-block DVE ops are emitted with a
  4-deep cross-block stagger so no DVE op consumes a result produced by the
  immediately preceding DVE op (result-forwarding stall avoidance).

Layout: HBM in is channel-major [img, 256, 3136] f32; tiles are DMA'd as
  [128 ch, pos] (contiguous), transposed on TensorE via identity matmul into
  PSUM [128 pos, 256 ch], copied to SBUF by ScalarE. The mask is stored
  pos-major [pos, 256] bf16 (no output transposes; one batched DMA per
  chunk); the host casts to f32 and transposes back to channel-major.

Sharding: pure data parallel, 4 images per core across 8 cores.
"""

import numpy as np

import concourse.bacc as bacc
import concourse.bass as bass
import concourse.mybir as mybir
from concourse import tile
from concourse.bass_utils import run_bass_kernel_spmd
from concourse.masks import make_identity

P = 128
C = 256                      # filter dim
N_CORES = 8


def _segments(s, e, hw):
    """Split flat-position range [s, e) into per-image contiguous pieces.

    Returns [(img, h0, h1, off)] with off the offset inside the chunk."""
    res = []
    off = 0
    while s < e:
        img = s // hw
        h0 = s - img * hw
        h1 = min(e - img * hw, hw)
        res.append((img, h0, h1, off))
        off += h1 - h0
        s = img * hw + h1
    return res


def _strip_self_waits(nc, engines=("DVE",)):
    """Remove semaphore waits where an instruction waits on its OWN
    engine's semaphore. Engines execute their stream in order, so a wait
    on a value that only earlier same-engine instructions increment is
    always satisfied; it just adds a sem round-trip to every dispatch."""
    n = 0
    for blk in nc.m.functions[0].blocks:
        for inst in blk.instructions:
            eng = str(getattr(inst, "engine", ""))
            si = getattr(inst, "sync_info", None)
            if si is None or not si.on_wait:
                continue
            eng_name = eng.split(".")[-1]
            if eng_name not in engines:
                continue
            keep = [w for w in si.on_wait
                    if not (w.ant_name or "").startswith(eng_name + "_")]
            if len(keep) != len(si.on_wait):
                n += len(si.on_wait) - len(keep)
                si.on_wait = keep
    return n


def _plan(nblk, chunk_blocks, taper):
    if taper and nblk >= 24:
        # small first/last chunks shrink the DMA ramp at kernel start/end
        plan = [2, 4]
        while sum(plan) + chunk_blocks <= nblk - 6:
            plan.append(chunk_blocks)
        rem = nblk - sum(plan)
        if rem > 4:
            plan.extend([rem - 2, 2])
        elif rem > 0:
            plan.append(rem)
    else:
        assert nblk % chunk_blocks == 0
        plan = [chunk_blocks] * (nblk // chunk_blocks)
    assert sum(plan) == nblk
    return plan


def build_nc(n_img=4, hw=3136, chunk_blocks=14, in_bufs=3, out_bufs=3,
             x_bufs=7, v_bufs=4, m_bufs=6, ps_bufs=4, taper=True,
             out_dt="bf16", s2_gp_mod=0, s4_gp_mod=0,
             mask_mode="dve_ge", pair_s0=False,
             strip_self_waits=()):
    tot = n_img * hw
    assert tot % P == 0
    nblk = tot // P
    plan = _plan(nblk, chunk_blocks, taper)
    if pair_s0:
        assert all(cb % 2 == 0 for cb in plan), plan
    f32 = mybir.dt.float32
    odt = {"bf16": mybir.dt.bfloat16, "fp8": mybir.dt.float8e4,
           "f32": f32}[out_dt]

    nc = bacc.Bacc("TRN2", target_bir_lowering=False, debug=False,
                   num_devices=N_CORES)
    x = nc.declare_dram_parameter("x", [n_img, C, hw], f32, isOutput=False)
    y = nc.declare_dram_parameter("y", [nblk, P, C], odt, isOutput=True)

    # chunk id / index-in-chunk / chunk start block for each global block
    chunk_of, j_of, c0_of = [], [], []
    b0 = 0
    for ci, cb in enumerate(plan):
        for j in range(cb):
            chunk_of.append(ci)
            j_of.append(j)
            c0_of.append(b0)
        b0 += cb

    with tile.TileContext(nc) as tc:
        with (
            tc.tile_pool(name="const", bufs=1) as const_pool,
            tc.tile_pool(name="inp", bufs=in_bufs) as in_pool,
            tc.tile_pool(name="outp", bufs=out_bufs) as out_pool,
            tc.tile_pool(name="xx", bufs=x_bufs) as x_pool,
            tc.tile_pool(name="vv", bufs=v_bufs) as v_pool,
            tc.tile_pool(name="m8", bufs=2 * m_bufs) as m_pool,
            tc.tile_pool(name="psin", bufs=ps_bufs, space="PSUM") as ps_pool,
        ):
            ident = const_pool.tile([P, P], f32)
            make_identity(nc, ident)

            ctx = {}        # per-inflight-block state, keyed by global blk
            chunk_tiles = {}

            def chunk_setup(g):
                ci = chunk_of[g]
                if j_of[g] == 0:
                    cb = plan[ci]
                    s = c0_of[g] * P
                    in_lo = in_pool.tile([P, cb * P], f32, tag="in_lo")
                    in_hi = in_pool.tile([P, cb * P], f32, tag="in_hi")
                    for (img, h0, h1, off) in _segments(s, s + cb * P, hw):
                        n = h1 - h0
                        nc.sync.dma_start(out=in_lo[:, off:off + n],
                                          in_=x[img, 0:P, h0:h1])
                        nc.sync.dma_start(out=in_hi[:, off:off + n],
                                          in_=x[img, P:C, h0:h1])
                    outc = out_pool.tile([P, cb, C], odt, tag="outc")
                    chunk_tiles[ci] = (in_lo, in_hi, outc)
                return chunk_tiles[ci]

            def s0(g):
                if pair_s0:
                    # two blocks share one full-bank PSUM tile + one Copy
                    if g % 2:
                        return
                    in_lo, in_hi, _ = chunk_setup(g)
                    ps = ps_pool.tile([P, 2, C], f32, tag="ps", name="ps")
                    x2 = x_pool.tile([P, 2, C], f32, tag="x", name="x")
                    for k in (0, 1):
                        j = j_of[g + k]
                        sl = slice(j * P, (j + 1) * P)
                        nc.tensor.transpose(ps[:, k, 0:P], in_lo[:, sl], ident)
                        nc.tensor.transpose(ps[:, k, P:C], in_hi[:, sl], ident)
                    nc.scalar.activation(x2, ps,
                                         mybir.ActivationFunctionType.Copy)
                    ctx[g] = {"x": x2[:, 0, :]}
                    ctx[g + 1] = {"x": x2[:, 1, :]}
                    return
                in_lo, in_hi, _ = chunk_setup(g)
                j = j_of[g]
                sl = slice(j * P, (j + 1) * P)
                ps = ps_pool.tile([P, C], f32, tag="ps", name="ps")
                nc.tensor.transpose(ps[:, 0:P], in_lo[:, sl], ident)
                nc.tensor.transpose(ps[:, P:C], in_hi[:, sl], ident)
                xsb = x_pool.tile([P, C], f32, tag="x", name="x")
                nc.scalar.activation(xsb, ps,
                                     mybir.ActivationFunctionType.Copy)
                ctx[g] = {"x": xsb}

            def s1(g):
                b = ctx[g]
                b["m1"] = m_pool.tile([P, 8], f32, tag="m1", name="m1")
                nc.vector.max(out=b["m1"], in_=b["x"])

            def s2(g):
                b = ctx[g]
                b["v"] = v_pool.tile([P, C], f32, tag="v", name="v")
                eng = (nc.gpsimd if s2_gp_mod and g % s2_gp_mod == 0
                       else nc.vector)
                eng.scalar_tensor_tensor(
                    out=b["v"], in0=b["x"], scalar=b["m1"][:, 7:8],
                    in1=b["x"], op0=mybir.AluOpType.is_lt,
                    op1=mybir.AluOpType.mult)

            def s3(g):
                b = ctx[g]
                b["m2"] = m_pool.tile([P, 8], f32, tag="m2", name="m2")
                nc.vector.max(out=b["m2"], in_=b["v"])

            def s4(g):
                b = ctx[g]
                ci = chunk_of[g]
                outc = chunk_tiles[ci][2]
                if mask_mode == "scalar_sign":
                    # Sign(t16 - x): -1/0 at selected (x >= t16), +1 below;
                    # the host decodes mask = (y <= 0).
                    nc.scalar.activation(outc[:, j_of[g], :], b["x"],
                                         mybir.ActivationFunctionType.Sign,
                                         bias=b["m2"][:, 7:8], scale=-1.0)
                else:
                    eng = (nc.gpsimd if s4_gp_mod and g % s4_gp_mod == 0
                           else nc.vector)
                    eng.tensor_scalar(out=outc[:, j_of[g], :], in0=b["x"],
                                      scalar1=b["m2"][:, 7:8], scalar2=None,
                                      op0=mybir.AluOpType.is_ge)
                if j_of[g] == plan[ci] - 1:
                    cb = plan[ci]
                    b0 = c0_of[g]
                    nc.sync.dma_start(
                        out=y[b0:b0 + cb].rearrange("b p c -> p b c"),
                        in_=outc[:, 0:cb, :])
                del ctx[g]

            stages = [s0, s1, s2, s3, s4]
            for step in range(nblk + 4):
                for d, st in enumerate(stages):
                    g = step - d
                    if 0 <= g < nblk:
                        st(g)
    nc.compile()
    if strip_self_waits:
        _strip_self_waits(nc, tuple(strip_self_waits))
    return nc


def _install_neff_cache():
    """Cache compiled NEFFs by BIR hash under /tmp so repeat runs skip
    the multi-minute neuronxcc compile."""
    import hashlib
    import os
    import shutil
    import concourse.bass2jax as b2j
    if getattr(b2j, "_topk_neff_cache_installed", False):
        return
    cache_dir = "/tmp/neff_cache"
    try:
        os.makedirs(cache_dir, exist_ok=True)
    except OSError:
        return
    orig_compile = b2j.compile_bir_kernel

    def cached_compile(ant_bir_str, compile_dir_path, neff_name):
        key = hashlib.sha256(ant_bir_str).hexdigest()[:32]
        cpath = os.path.join(cache_dir, key + ".neff")
        if os.path.exists(cpath):
            dst = os.path.join(compile_dir_path, neff_name)
            shutil.copy(cpath, dst)
            return dst
        out = orig_compile(ant_bir_str, compile_dir_path, neff_name=neff_name)
        try:
            shutil.copy(out, cpath)
        except OSError:
            pass
        return out

    b2j.compile_bir_kernel = cached_compile
    b2j._topk_neff_cache_installed = True


_install_neff_cache()

_NC_CACHE = {}


def _get_nc(n_img, hw, chunk_blocks, **kw):
    key = (n_img, hw, chunk_blocks, tuple(sorted(kw.items())))
    if key not in _NC_CACHE:
        _NC_CACHE[key] = build_nc(n_img, hw, chunk_blocks, **kw)
    return _NC_CACHE[key]


KERNEL_KW = dict(mask_mode="scalar_sign", pair_s0=True)


def make_in_maps(x, n_img, kw=KERNEL_KW):
    return [{"x": np.ascontiguousarray(x[i * n_img:(i + 1) * n_img])}
            for i in range(N_CORES)]


def unshard(core_outputs, n_img=4, hw=3136, kw=KERNEL_KW):
    """[nblk, 128, C] per-core pos-major device outputs -> [B, C, H, W]."""
    sign = kw.get("mask_mode", "dve_ge") == "scalar_sign"
    parts = []
    for yc in core_outputs:
        yc = np.asarray(yc).reshape(n_img, hw, C).astype(np.float32)
        if sign:
            yc = (yc <= 0.0).astype(np.float32)
        parts.append(yc.transpose(0, 2, 1))          # -> [n_img, C, hw]
    y = np.concatenate(parts, axis=0)
    B = len(core_outputs) * n_img
    s = int(round(hw ** 0.5))
    return np.ascontiguousarray(y.reshape(B, C, s, s))


def kernel(activations: np.ndarray) -> np.ndarray:
    B, Cin, H, W = activations.shape
    assert (B, Cin, H, W) == (32, 256, 56, 56)
    hw = H * W
    n_img = B // N_CORES
    x = np.ascontiguousarray(activations, dtype=np.float32).reshape(B, Cin, hw)
    nc = _get_nc(n_img, hw, 14, **KERNEL_KW)
    in_maps = make_in_maps(x, n_img)
    res = run_bass_kernel_spmd(nc, in_maps, list(range(N_CORES)))
    return unshard([res.results[i]["y"] for i in range(N_CORES)],
                   n_img=n_img, hw=hw)
